# revision 51
# baseline (speedup 1.0000x reference)
"""Trainium2 Bass kernel for nn_EnhancedSNNCifar (8-core data parallel).

Strategy
--------
Pure data parallel: batch 128 -> 16 images per NeuronCore, all weights
replicated. BN uses local-batch statistics (per-shard, as sanctioned by
the sharding spec) so no collectives are needed.

Per-core kernel:
- Channels on partitions; when C < 128, image-groups are packed into
  the spare partition blocks. Group/slot labels get permuted by each
  conv's PSUM col-block assignment; the final permutation is undone on
  the host.
- All matmul operands (weights, spikes, conv1 im2col) are fp16;
  accumulation stays fp32 in PSUM.
- Convs: 9 shifted matmuls accumulating in PSUM over padded SBUF spike
  slots. Small-C layers use TensorE sub-array tiling (tile_position),
  up to 16 concurrent (K=32,M=32) tiles.
- conv1 exploits the T-broadcast of the input: computed once (im2col
  K=27).
- All pre-BN conv outputs (pb buffers) live in SBUF as fp16 — no DRAM
  round trip. Eviction is an ACT Copy (PSUM->pb fp16, accum_out =
  per-channel sums) plus an ACT Square (PSUM->scratch, accum_out =
  sumsq).
- LIF(L) and conv(L+1) are interleaved per timestep through
  double-buffered spike slots, so VectorE (LIF) overlaps TensorE
  (conv) across the layer boundary.
- LIF runs in "p-space" (p_t = v_t * 2^t), all fp16:
    p_t   = x_t*(inv*2^(t-1)) + shift*2^(t-1) + pk_{t-1}  (AFFINE_THEN_ADD)
    spike = p_t >= 2^t                                    (tensor_scalar is_ge)
    pk_t  = select(p_t < 2^t, p_t, 0)                     (TENSOR_MASK)
  MaxPool folds into the spike op (spike of max(p) over the 2x2
  window).
"""
import os
import numpy as np

import concourse.bass as bass
import concourse.tile as tile
import concourse.mybir as mybir
from concourse import bacc

F32 = mybir.dt.float32
F16 = mybir.dt.float16
Alu = mybir.AluOpType
Act = mybir.ActivationFunctionType

T = 8
N_CORES = 8
N_LOC = 16
EPS = 1e-5
DEBUG = bool(os.environ.get("SNN_DEBUG"))

LCFG = [
    dict(name='2', ci=32, co=32, h=32, pool=True),
    dict(name='3', ci=32, co=64, h=16, pool=False),
    dict(name='4', ci=64, co=64, h=16, pool=True),
    dict(name='5', ci=64, co=128, h=8, pool=False),
    dict(name='6', ci=128, co=128, h=8, pool=True),
]
for L in LCFG:
    L['gi'] = 128 // L['ci']
    L['si'] = N_LOC // L['gi']
    L['go'] = 128 // L['co']
    L['so_cnt'] = N_LOC // L['go']


def _slot_maps():
    cur = [[4 * q + g for q in range(4)] for g in range(4)]
    for L in LCFG:
        gi, si, go = L['gi'], L['si'], L['go']
        nxt = [[None] * (N_LOC // go) for _ in range(go)]
        for g in range(gi):
            for s in range(si):
                j = s % go
                so = g * (si // go) + s // go
                nxt[j][so] = cur[g][s]
        cur = nxt
    return cur[0]


FINAL_SLOTS = _slot_maps()


def build_module():
    nc = bacc.Bacc(trn_type="TRN2", num_devices=N_CORES, name="snn",
                   dynamic_dma_scratch_size=2048)

    D = {}
    D['xpad'] = nc.dram_tensor("xpad", [3, N_LOC, 34, 34], F16,
                               kind="ExternalInput").ap()
    D['w1'] = nc.dram_tensor("w1im", [9, 3 * 32], F16,
                             kind="ExternalInput").ap()
    D['wd'] = {}
    D['bn'] = {}
    for L in LCFG:
        s = L['name']
        if s == '2':
            D['wd'][s] = nc.dram_tensor("w2", [96, 3, 32], F16,
                                        kind="ExternalInput").ap()
        else:
            D['wd'][s] = nc.dram_tensor(f"w{s}", [L['ci'], 9, L['co']],
                                        F16, kind="ExternalInput").ap()
    for s in ['1', '2', '3', '4', '5', '6']:
        D['bn'][s] = nc.dram_tensor(f"bn{s}", [128, 3], F32,
                                    kind="ExternalInput").ap()
    D['fc1w'] = nc.dram_tensor("fc1w", [128, 16, 128], F16,
                               kind="ExternalInput").ap()
    D['fc1b'] = nc.dram_tensor("fc1b", [128, 1], F32,
                               kind="ExternalInput").ap()
    D['fc2w'] = nc.dram_tensor("fc2w", [128, 10], F16,
                               kind="ExternalInput").ap()
    D['fc2b'] = nc.dram_tensor("fc2b", [10, 1], F32,
                               kind="ExternalInput").ap()
    D['out'] = nc.dram_tensor("out", [10, N_LOC], F32,
                              kind="ExternalOutput").ap()
    if DEBUG:
        D['o2dbg'] = nc.dram_tensor("o2dbg", [10, 128], F32,
                                    kind="ExternalOutput").ap()
        D['h1dbg'] = nc.dram_tensor("h1dbg", [128, 128], F32,
                                    kind="ExternalOutput").ap()
    # local-batch BN: stats over this core's 16-image shard only
    D['cnt'] = {'1': N_LOC * 1024.0, '2': 8 * N_LOC * 1024.0,
                '3': 8 * N_LOC * 256.0, '4': 8 * N_LOC * 256.0,
                '5': 8 * N_LOC * 64.0, '6': 8 * N_LOC * 64.0}

    from contextlib import ExitStack
    with tile.TileContext(nc) as tc:
        with ExitStack() as es:
            build_body(nc, tc, es, D)
    nc.compile()
    return nc


def build_body(nc, tc, es, D):
    glob = es.enter_context(tc.tile_pool(name="glob", bufs=1))
    ppool = es.enter_context(tc.tile_pool(name="ppool", bufs=2))
    mxp = es.enter_context(tc.tile_pool(name="mxp", bufs=1))
    sqp = es.enter_context(tc.tile_pool(name="sqp", bufs=1))
    spp = es.enter_context(tc.tile_pool(name="spp", bufs=2))
    pbp = es.enter_context(tc.tile_pool(name="pbp", bufs=1))
    psum = es.enter_context(tc.tile_pool(name="psum", bufs=4, space="PSUM"))

    AB = {}
    for s in ['1', '2', '3', '4', '5', '6']:
        AB[s] = (glob.tile([128, 8], F32, tag=f"A{s}", name=f"A{s}"),
                 glob.tile([128, 8], F32, tag=f"B{s}", name=f"B{s}"))
    pow2row = glob.tile([128, 8], F32, tag="pow2", name="pow2row")
    for t in range(T):
        nc.vector.memset(pow2row[:, t:t + 1], float(2.0 ** (t - 1)))

    # ---- preload all weights ----
    w1_sb = glob.tile([9, 3 * 32], F16, tag="w1", name="w1")
    nc.sync.dma_start(w1_sb[:], D['w1'][:])
    WS = {}
    for L in LCFG:
        s = L['name']
        ci, gi = L['ci'], L['gi']
        if s == '2':
            w_sb = glob.tile([96, 96], F16, tag="w2", name="w2")
            nc.sync.dma_start(
                w_sb[:], D['wd'][s][:].rearrange("k dx co -> k (dx co)"))
        else:
            w_sb = glob.tile([128, 9 * L['co']], F16, tag=f"w{s}",
                             name=f"w{s}")
            src = D['wd'][s][:].rearrange("ci k co -> ci (k co)")
            for g in range(gi):
                nc.sync.dma_start(w_sb[g * ci:(g + 1) * ci, :], src)
        WS[s] = w_sb
    fc1w = glob.tile([128, 16 * 128], F16, tag="fc1w", name="fc1w")
    nc.sync.dma_start(fc1w[:], D['fc1w'][:].rearrange("c s o -> c (s o)"))
    fc1b = glob.tile([128, 1], F32, tag="fc1b", name="fc1b")
    nc.sync.dma_start(fc1b[:], D['fc1b'][:])
    fc2w = glob.tile([128, 10], F16, tag="fc2w", name="fc2w")
    nc.sync.dma_start(fc2w[:], D['fc2w'][:])
    fc2b = glob.tile([10, 1], F32, tag="fc2b", name="fc2b")
    nc.sync.dma_start(fc2b[:], D['fc2b'][:])

    def evict(psrc, ddst, ssum_col, ssq_col):
        """ACT Copy psum->pb fp16 (+sum), ACT Square psum->scratch
        (+sumsq)."""
        npart = psrc.shape[0]
        fd = psrc.free_size()
        sq = sqp.tile([128, 1024], F32, tag="sq", name="sq")
        nc.scalar.activation(ddst, psrc, Act.Copy, accum_out=ssum_col)
        nc.scalar.activation(sq[0:npart, 0:fd], psrc, Act.Square,
                             accum_out=ssq_col)

    def finalize_bn(s, ssum_strip, ssq_strip, go, co):
        bnp = glob.tile([128, 3], F32, tag=f"bn{s}", name=f"bnp{s}")
        nc.sync.dma_start(bnp[:], D['bn'][s][:])
        tot = glob.tile([128, 2], F32, tag=f"tot{s}", name=f"tot{s}")
        nc.vector.reduce_sum(tot[:, 0:1], ssum_strip[:],
                             axis=mybir.AxisListType.X)
        nc.vector.reduce_sum(tot[:, 1:2], ssq_strip[:],
                             axis=mybir.AxisListType.X)
        if go > 1:
            # cross-partition-base TT is illegal: stage the blocks into
            # base-aligned columns, add columns, then broadcast back.
            fold = glob.tile([128, 2 * 4], F32, tag=f"fold{s}",
                             name=f"fold{s}")
            for g in range(1, go):
                nc.vector.tensor_copy(fold[0:co, 2 * g:2 * g + 2],
                                      tot[g * co:(g + 1) * co, :])
            for g in range(1, go):
                nc.vector.tensor_tensor(tot[0:co, :], tot[0:co, :],
                                        fold[0:co, 2 * g:2 * g + 2],
                                        Alu.add)
            for g in range(1, go):
                nc.vector.tensor_copy(tot[g * co:(g + 1) * co, :],
                                      tot[0:co, :])
        sc = glob.tile([128, 6], F32, tag=f"sc{s}", name=f"sc{s}")
        m, ex2, var, inv, sh, tmp = [sc[:, i:i + 1] for i in range(6)]
        icnt = 1.0 / D['cnt'][s]
        nc.vector.tensor_scalar(m, tot[:, 0:1], icnt, None, Alu.mult)
        nc.vector.tensor_scalar(ex2, tot[:, 1:2], icnt, None, Alu.mult)
        nc.vector.tensor_tensor(tmp, m, m, Alu.mult)
        nc.vector.tensor_tensor(var, ex2, tmp, Alu.subtract)
        nc.vector.tensor_scalar(var, var, EPS, None, Alu.add)
        nc.scalar.activation(tmp, var, Act.Sqrt)
        nc.vector.reciprocal(var, tmp)
        nc.vector.tensor_tensor(inv, var, bnp[:, 0:1], Alu.mult)
        nc.vector.tensor_tensor(sh, bnp[:, 2:3], m, Alu.subtract)
        nc.vector.tensor_tensor(sh, sh, inv, Alu.mult)
        nc.vector.tensor_tensor(sh, sh, bnp[:, 1:2], Alu.add)
        A, B = AB[s]
        nc.vector.tensor_scalar(A[:], pow2row[:], inv, None, Alu.mult)
        nc.vector.tensor_scalar(B[:], pow2row[:], sh, None, Alu.mult)

    def sp_slots(L_next, padded=True):
        """Two rotating per-t spike slot tiles, halos pre-zeroed."""
        h = L_next['h']
        hp = h + 2 if padded else h
        si = L_next['si']
        slots = []
        for b in range(2):
            tl = spp.tile([128, si, hp, hp], F16, tag=f"sp{L_next['name']}",
                          name=f"sp{L_next['name']}_{b}")
            if padded:
                nc.vector.memset(tl[:, :, 0:1, :], 0.0)
                nc.vector.memset(tl[:, :, hp - 1:hp, :], 0.0)
                nc.vector.memset(tl[:, :, :, 0:1], 0.0)
                nc.vector.memset(tl[:, :, :, hp - 1:hp], 0.0)
            slots.append(tl)
        return slots

    last_pst = [None]

    def pe_tickle(src_tile):
        """Tiny matmul chained to src_tile, accumulating garbage into
        the previous (already-evicted, about-to-be-recycled) PSUM tile:
        keeps the PE HAM activity window busy during vector-dominated
        stretches so conv bursts run at the warm 2.4 GHz clock instead
        of re-throttled 1.2. The target region is never read before
        its next start=True clear, so the garbage is inert."""
        if last_pst[0] is None:
            return
        nc.tensor.matmul(last_pst[0][0:32, 0:4], w1_sb[:, 0:32],
                         src_tile[0:9, 0:4], start=False, stop=False,
                         skip_group_check=True)

    def lif_step(s_lif, t, xin, fd, pk, dst, pool_shape):
        """One LIF timestep: returns new pk tile (or None at t=T-1).
        xin: [128, fd] AP of pre-BN x_t. dst: spike destination AP
        (padded interior view already sliced). pool_shape: None or
        (so, h) to maxpool p before thresholding. The recurrence runs
        on VectorE with standard fp16 ops; spike generation (and
        pooling) is offloaded to GpSimd for fd >= 2048."""
        A, B = AB[s_lif]
        th = float(2.0 ** t)
        p = ppool.tile([128, fd], F16, tag="p", bufs=3, name="p")
        if t == 0:
            nc.vector.tensor_scalar(p[:], xin, A[:, 0:1], B[:, 0:1],
                                    Alu.mult, Alu.add)
        else:
            tmp = ppool.tile([128, fd], F16, tag="p", bufs=3, name="tmp")
            nc.vector.tensor_scalar(tmp[:], xin, A[:, t:t + 1],
                                    B[:, t:t + 1], Alu.mult, Alu.add)
            nc.vector.tensor_tensor(p[:], tmp[:], pk[:], Alu.add)
        pe_tickle(p)
        if pool_shape is not None:
            so, h = pool_shape
            pv = p[:].rearrange("c (so y x) -> c so y x", so=so, y=h, x=h)
            mx = mxp.tile([128, so * h * (h // 2)], F16, tag="mx", name="mx")
            mxv = mx[:].rearrange("c (so y x) -> c so y x",
                                  so=so, y=h, x=h // 2)
            nc.vector.tensor_tensor(mxv[:], pv[:, :, :, 0:h:2],
                                    pv[:, :, :, 1:h:2], Alu.max)
            myv = mxv[:, :, 0:h:2, :]
            nc.vector.tensor_tensor(myv, mxv[:, :, 0:h:2, :],
                                    mxv[:, :, 1:h:2, :], Alu.max)
            src = myv
        else:
            src = p[:]
        nc.vector.tensor_scalar(dst, src, th, None, Alu.is_ge)
        if t < T - 1:
            nsp = ppool.tile([128, fd], F16, tag="p", bufs=3, name="nsp")
            nc.vector.tensor_scalar(nsp[:], p[:], th, None, Alu.is_lt)
            pk2 = ppool.tile([128, fd], F16, tag="p", bufs=3, name="pk")
            nc.vector.tensor_tensor(pk2[:], p[:], nsp[:], Alu.mult)
            pe_tickle(pk2)
            return pk2
        return None

    def conv_t(L, sp_t, w_sb, t, pb_tile, ssum, ssq, ecol):
        """One timestep of conv L: matmuls from spike slot sp_t into
        PSUM, evict into pb_tile[:, t, ...] (+stats cols from ecol)."""
        s = L['name']
        ci, co, gi, go, h = L['ci'], L['co'], L['gi'], L['go'], L['h']
        hw = h * h
        ipc = max(1, 512 // hw)

        def one_mm(g, j, chunk, k, out_sl, start, stop):
            dy, dx = k // 3, k % 3
            if ipc == 1:
                nr = 512 // h
                r0 = chunk * nr
                rhs = sp_t[ci * g:ci * g + ci, j,
                           r0 + dy:r0 + dy + nr, dx:dx + h]
            else:
                s0 = j + go * chunk * ipc
                rhs = sp_t[ci * g:ci * g + ci,
                           s0:s0 + go * (ipc - 1) + 1:go,
                           dy:dy + h, dx:dx + h]
            tp = None
            if ci < 128 or co < 128:
                tp = (ci * g, co * j)
            nc.tensor.matmul(
                out_sl, w_sb[ci * g:ci * g + ci, co * k:co * k + co],
                rhs, start=start, stop=stop, tile_position=tp,
                skip_group_check=True)

        def do_evict(dst_flat, pslice):
            evict(pslice, dst_flat,
                  ssum[:, ecol[0]:ecol[0] + 1],
                  ssq[:, ecol[0]:ecol[0] + 1])
            ecol[0] += 1

        if gi == 1:                       # L6: one tile, 2 chunks
            pst = psum.tile([128, 1024], F32, tag="ps", name="ps")
            for k in range(9):
                for chunk in range(2):
                    one_mm(0, 0, chunk, k,
                           pst[:, 512 * chunk:512 * chunk + 512],
                           k == 0, k == 8)
            do_evict(pb_tile[:, t].rearrange("c s y x -> c (s y x)"),
                     pst[:])
            last_pst[0] = pst
        elif go == 1:                     # L5: 2 row tiles
            pst = psum.tile([128, 1024], F32, tag="ps", name="ps")
            for k in range(9):
                for g in range(gi):
                    one_mm(g, 0, 0, k,
                           pst[:, 512 * g:512 * g + 512],
                           k == 0, k == 8)
            do_evict(pb_tile[:, t].rearrange("c s y x -> c (s y x)"),
                     pst[:])
            last_pst[0] = pst
        elif ci == 32:                    # L3: 8 tiles (2q x 2u x 2j)
            psts = [psum.tile([128, 1024], F32, tag="ps", name="ps")
                    for _ in range(2)]
            for k in range(9):
                for q in range(2):
                    for u in range(2):
                        for j in range(go):
                            one_mm(2 * q + u, j, 0, k,
                                   psts[q][64 * j:64 * j + 64,
                                           512 * u:512 * u + 512],
                                   k == 0, k == 8)
            for q in range(2):
                do_evict(
                    pb_tile[:, t, 4 * q:4 * q + 4].rearrange(
                        "c s y x -> c (s y x)"),
                    psts[q][:])
            last_pst[0] = psts[1]
        else:                             # L4: 4 tiles (2g x 2j), 2v
            psts = [psum.tile([128, 1024], F32, tag="ps", name="ps")
                    for _ in range(2)]
            for k in range(9):
                for v in range(2):
                    for g in range(gi):
                        for j in range(go):
                            one_mm(g, j, v, k,
                                   psts[g][64 * j:64 * j + 64,
                                           512 * v:512 * v + 512],
                                   k == 0, k == 8)
            for g in range(2):
                do_evict(
                    pb_tile[:, t, 4 * g:4 * g + 4].rearrange(
                        "c s y x -> c (s y x)"),
                    psts[g][:])
            last_pst[0] = psts[1]

    # ================= Stage 1: conv1 + BN1 =================
    y1 = glob.tile([128, 4, 32, 32], F16, tag="y1", name="y1")
    ssum1 = glob.tile([128, 4], F32, tag="ssum1", name="ssum1")
    ssq1 = glob.tile([128, 4], F32, tag="ssq1", name="ssq1")
    nc.vector.memset(ssum1[:], 0.0)
    nc.vector.memset(ssq1[:], 0.0)

    # dy-stacked input: partition block d holds xpad shifted down by d,
    # so one K=9 matmul contracts over (channel, dy); dx becomes 3
    # accumulation steps via free-dim shifted views.
    xpad = D['xpad']
    xs3 = pbp.tile([9, N_LOC, 32, 34], F16, tag="pbA", name="xs3")
    for dy in range(3):
        nc.sync.dma_start(xs3[3 * dy:3 * dy + 3, :, :, :],
                          xpad[:, :, dy:dy + 32, :])
    for q in range(4):
        pst = psum.tile([128, 1024], F32, tag="ps", name="ps")
        for dx in range(3):
            for hh in range(2):
                for r in range(4):
                    nc.tensor.matmul(
                        pst[32 * r:32 * r + 32, 512 * hh:512 * hh + 512],
                        w1_sb[:, 32 * dx:32 * dx + 32],
                        xs3[:, 4 * q + r, 16 * hh:16 * hh + 16,
                            dx:dx + 32],
                        start=(dx == 0), stop=(dx == 2),
                        tile_position=(0, 32 * r), skip_group_check=True)
        sq = sqp.tile([128, 1024], F32, tag="sq", name="sq")
        nc.scalar.activation(
            y1[:, q, :, :].rearrange("c y x -> c (y x)"),
            pst[:], Act.Copy, accum_out=ssum1[:, q:q + 1])
        nc.scalar.activation(sq[:], pst[:], Act.Square,
                             accum_out=ssq1[:, q:q + 1])
        last_pst[0] = pst
    finalize_bn('1', ssum1, ssq1, 4, 32)

    # ============ Stage 2: LIF1 + conv2 (per-t interleaved) ============
    # conv2 is dy-stacked like conv1: LIF1 spikes are replicated onto
    # 3 partition blocks (y-shifted) via 12 SBUF->SBUF DMAs per t, so
    # each output needs only 3 K=96 accumulation steps instead of 9
    # K=32 ones (3x less PE streaming).
    l2 = LCFG[0]
    ssum2 = glob.tile([128, 32], F32, tag="ssum2", name="ssum2")
    ssq2 = glob.tile([128, 32], F32, tag="ssq2", name="ssq2")
    nc.vector.memset(ssum2[:], 0.0)
    nc.vector.memset(ssq2[:], 0.0)
    pb2 = pbp.tile([128, T, 4, 32, 32], F16, tag="pbA", name="pb2")
    xs2 = pbp.tile([96, N_LOC, 32, 34], F16, tag="pbB", name="xs2")
    w2s = WS['2']

    stg_slots = sp_slots(l2, padded=True)
    y1flat = y1[:].rearrange("c s y x -> c (s y x)")
    pk = None
    ecol2 = [0]
    for t in range(T):
        slot = stg_slots[t % 2]
        pk = lif_step('1', t, y1flat, 4096, pk,
                      slot[:, :, 1:33, 1:33], None)
        for dy in range(3):
            for g in range(4):
                nc.sync.dma_start(
                    xs2[32 * dy:32 * dy + 32, 4 * g:4 * g + 4, :, :],
                    slot[32 * g:32 * g + 32, :, dy:dy + 32, :])
        psts = [psum.tile([128, 1024], F32, tag="ps", name="ps")
                for _ in range(4)]
        for g in range(4):
            for j in range(4):
                for hh in range(2):
                    for dx in range(3):
                        nc.tensor.matmul(
                            psts[g][32 * j:32 * j + 32,
                                    512 * hh:512 * hh + 512],
                            w2s[:, 32 * dx:32 * dx + 32],
                            xs2[:, 4 * g + j, 16 * hh:16 * hh + 16,
                                dx:dx + 32],
                            start=(dx == 0), stop=(dx == 2),
                            tile_position=(0, 32 * j),
                            skip_group_check=True)
        for g in range(4):
            evict(psts[g][:],
                  pb2[:, t, g].rearrange("c y x -> c (y x)"),
                  ssum2[:, ecol2[0]:ecol2[0] + 1],
                  ssq2[:, ecol2[0]:ecol2[0] + 1])
            ecol2[0] += 1
        last_pst[0] = psts[3]
    finalize_bn('2', ssum2, ssq2, 4, 32)

    # ============ Chain: LIF(prev) + conv(L) per-t ============
    N_EV = {'3': 16, '4': 16, '5': 8, '6': 8}
    PB_TAG = {'3': 'pbB', '4': 'pbA', '5': 'pbB', '6': 'pbA'}
    prev_L = l2
    prev_pb = pb2
    for idx in range(1, len(LCFG)):
        nxt = LCFG[idx]
        sn, sp_ = nxt['name'], prev_L['name']
        slots = sp_slots(nxt, padded=True)
        ssum_n = glob.tile([128, N_EV[sn]], F32, tag=f"ssum{sn}",
                           name=f"ssum{sn}")
        ssq_n = glob.tile([128, N_EV[sn]], F32, tag=f"ssq{sn}",
                          name=f"ssq{sn}")
        nc.vector.memset(ssum_n[:], 0.0)
        nc.vector.memset(ssq_n[:], 0.0)
        pb_n = pbp.tile([128, T, nxt['so_cnt'], nxt['h'], nxt['h']], F16,
                        tag=PB_TAG[sn], name=f"pb{sn}")

        so_p, h_p = prev_L['so_cnt'], prev_L['h']
        fd_p = so_p * h_p * h_p
        ho = h_p // 2 if prev_L['pool'] else h_p
        pool_shape = (so_p, h_p) if prev_L['pool'] else None
        pk = None
        ecol = [0]
        for t in range(T):
            slot = slots[t % 2]
            xin = prev_pb[:, t].rearrange("c s y x -> c (s y x)")
            pk = lif_step(sp_, t, xin, fd_p, pk,
                          slot[:, :, 1:ho + 1, 1:ho + 1], pool_shape)
            conv_t(nxt, slot, WS[sn], t, pb_n, ssum_n, ssq_n, ecol)
        finalize_bn(sn, ssum_n, ssq_n, nxt['go'], nxt['co'])
        prev_L = nxt
        prev_pb = pb_n

    # ===== LIF6 -> s6, fc1 matmuls interleaved per-t =====
    bf1 = glob.tile([128, 8], F32, tag="bf1", name="bf1")
    bf2 = glob.tile([10, 8], F32, tag="bf2", name="bf2")
    nc.vector.tensor_scalar(bf1[:], pow2row[:], fc1b[:, 0:1], None, Alu.mult)
    nc.vector.tensor_scalar(bf2[:], pow2row[0:10, :], fc2b[:, 0:1], None,
                            Alu.mult)

    s6 = glob.tile([128, T, 16, 4, 4], F16, tag="s6", name="s6")
    s6v = s6[:].rearrange("c t s y x -> c t s (y x)")
    pstf = psum.tile([128, 1024], F32, tag="ps", name="psfc")
    pfc = pstf[:, 0:128]
    pk = None
    for t in range(T):
        xin = prev_pb[:, t].rearrange("c s y x -> c (s y x)")
        pk = lif_step('6', t, xin, 1024, pk,
                      s6[:, t].rearrange("c s y x -> c (s y x)"),
                      (16, 8))
        for pos in range(16):
            nc.tensor.matmul(pfc[:, 16 * t:16 * t + 16],
                             fc1w[:, pos * 128:(pos + 1) * 128],
                             s6v[:, t, :, pos],
                             start=(pos == 0), stop=(pos == 15),
                             skip_group_check=True)
    if DEBUG:
        h1 = glob.tile([128, 128], F32, tag="h1", name="h1")
        nc.scalar.activation(h1[:], pfc, Act.Copy)
        nc.sync.dma_start(D['h1dbg'], h1[:])

    # ===== fc1 LIF (reads PSUM directly) =====
    h1s = glob.tile([128, 128], F16, tag="h1s", name="h1s")
    pk = None
    for t in range(T):
        th = float(2.0 ** t)
        p = ppool.tile([128, 16], F32, tag="pf", bufs=3, name="pf")
        xin = pfc[:, 16 * t:16 * t + 16]
        if t == 0:
            nc.vector.tensor_scalar(p[:], xin, 0.5, bf1[:, 0:1],
                                    Alu.mult, Alu.add)
        else:
            tmp = ppool.tile([128, 16], F32, tag="pf", bufs=3, name="pft")
            nc.vector.tensor_scalar(tmp[:], xin, float(2.0 ** (t - 1)),
                                    bf1[:, t:t + 1], Alu.mult, Alu.add)
            nc.vector.tensor_tensor(p[:], tmp[:], pk[:], Alu.add)
        nc.vector.tensor_scalar(h1s[:, 16 * t:16 * t + 16], p[:], th, None,
                                Alu.is_ge)
        if t < T - 1:
            pk2 = ppool.tile([128, 16], F32, tag="pf", bufs=3, name="pfk")
            nc.vector.scalar_tensor_tensor(pk2[:], p[:], th, p[:],
                                           Alu.is_lt, Alu.mult)
            pk = pk2

    pst2 = psum.tile([128, 1024], F32, tag="ps", name="ps2")
    po = pst2[0:10, 0:128]
    nc.tensor.matmul(po, fc2w[:], h1s[:], start=True, stop=True)
    if DEBUG:
        o2d = glob.tile([10, 128], F32, tag="o2", name="o2")
        nc.scalar.activation(o2d[:], po, Act.Copy)
        nc.sync.dma_start(D['o2dbg'], o2d[:])

    # ===== fc2 LIF + rate accumulation (reads PSUM directly) =====
    oacc = glob.tile([10, 16], F32, tag="oaccA", name="oacc")
    pk = None
    for t in range(T):
        th = float(2.0 ** t)
        p = ppool.tile([10, 16], F32, tag="pf", bufs=3, name="pg")
        xin = po[:, 16 * t:16 * t + 16]
        if t == 0:
            nc.vector.tensor_scalar(p[:], xin, 0.5, bf2[:, 0:1],
                                    Alu.mult, Alu.add)
        else:
            tmp = ppool.tile([10, 16], F32, tag="pf", bufs=3, name="pgt")
            nc.vector.tensor_scalar(tmp[:], xin, float(2.0 ** (t - 1)),
                                    bf2[:, t:t + 1], Alu.mult, Alu.add)
            nc.vector.tensor_tensor(p[:], tmp[:], pk[:], Alu.add)
        spk = glob.tile([10, 16], F32, tag=f"spk{t % 2}", name="spk")
        nc.vector.tensor_scalar(spk[:], p[:], th, None, Alu.is_ge)
        if t == 0:
            nc.vector.tensor_scalar(oacc[:], spk[:], 1.0 / T, None, Alu.mult)
        else:
            oacc2 = glob.tile([10, 16], F32, tag=f"oacc{t % 2}",
                              name="oacc2")
            nc.vector.scalar_tensor_tensor(oacc2[:], spk[:], 1.0 / T,
                                           oacc[:], Alu.mult, Alu.add)
            oacc = oacc2
        if t < T - 1:
            pk2 = ppool.tile([10, 16], F32, tag="pf", bufs=3, name="pgk")
            nc.vector.scalar_tensor_tensor(pk2[:], p[:], th, p[:],
                                           Alu.is_lt, Alu.mult)
            pk = pk2

    nc.sync.dma_start(D['out'], oacc[:])


# ===================== host side =====================
_CACHE = {}


def _get_module():
    if "nc" not in _CACHE:
        _CACHE["nc"] = build_module()
    return _CACHE["nc"]


def _prep_inputs(inputs):
    x = np.ascontiguousarray(np.asarray(inputs['x'], np.float32))
    N = x.shape[0]
    n_loc = N // N_CORES

    w1 = np.asarray(inputs['w1'], np.float32)
    w1im = np.zeros((9, 3 * 32), np.float32)
    for dy in range(3):
        for dx in range(3):
            for c in range(3):
                w1im[dy * 3 + c, 32 * dx:32 * dx + 32] = w1[:, c, dy, dx]

    shared = {"w1im": w1im.astype(np.float16)}
    for L in LCFG:
        s = L['name']
        w = np.asarray(inputs['w' + s], np.float32)
        if s == '2':
            # [co, ci, dy, dx] -> [dy*32+ci, dx, co]
            shared["w2"] = np.ascontiguousarray(
                w.transpose(2, 1, 3, 0).reshape(96, 3, 32)
            ).astype(np.float16)
        else:
            shared[f"w{s}"] = np.ascontiguousarray(
                w.transpose(1, 2, 3, 0).reshape(L['ci'], 9, L['co'])
            ).astype(np.float16)
    for s, go in [('1', 4), ('2', 4), ('3', 2), ('4', 2), ('5', 1),
                  ('6', 1)]:
        g = np.tile(np.asarray(inputs['g' + s], np.float32), go)
        be = np.tile(np.asarray(inputs['be' + s], np.float32), go)
        b = np.tile(np.asarray(inputs['b' + s], np.float32), go)
        shared[f"bn{s}"] = np.ascontiguousarray(np.stack([g, be, b], axis=1))
    fc1w = np.asarray(inputs['fc1_w'], np.float32)
    shared["fc1w"] = np.ascontiguousarray(
        fc1w.reshape(128, 128, 16).transpose(1, 2, 0)).astype(np.float16)
    shared["fc1b"] = np.asarray(inputs['fc1_b'], np.float32).reshape(128, 1)
    shared["fc2w"] = np.ascontiguousarray(
        np.asarray(inputs['fc2_w'], np.float32).T).astype(np.float16)
    shared["fc2b"] = np.asarray(inputs['fc2_b'], np.float32).reshape(10, 1)

    in_maps = []
    for c in range(N_CORES):
        xs = x[c * n_loc:(c + 1) * n_loc]
        xp = np.zeros((3, n_loc, 34, 34), np.float16)
        xp[:, :, 1:33, 1:33] = xs.transpose(1, 0, 2, 3).astype(np.float16)
        m = dict(shared)
        m["xpad"] = np.ascontiguousarray(xp)
        in_maps.append(m)
    return in_maps


def kernel(**inputs) -> np.ndarray:
    from concourse.bass_utils import run_bass_kernel_spmd
    nc = _get_module()
    in_maps = _prep_inputs(inputs)
    res = run_bass_kernel_spmd(nc, in_maps, core_ids=list(range(N_CORES)))
    N = np.asarray(inputs['x']).shape[0]
    n_loc = N // N_CORES
    out = np.zeros((N, 10), np.float32)
    for c in range(N_CORES):
        o = res.results[c]["out"]
        for s_idx in range(n_loc):
            out[c * n_loc + FINAL_SLOTS[s_idx], :] = o[:, s_idx]
    return out


if __name__ == "__main__":
    _get_module()
    print("module built OK")


# revision 55
# speedup vs baseline: 1.0176x; 1.0176x over previous
"""Trainium2 Bass kernel for nn_EnhancedSNNCifar (8-core data parallel).

Strategy
--------
Pure data parallel: batch 128 -> 16 images per NeuronCore, all weights
replicated. BN uses local-batch statistics (per-shard, as sanctioned by
the sharding spec) so no collectives are needed.

Per-core kernel:
- Channels on partitions; when C < 128, image-groups are packed into
  the spare partition blocks. Group/slot labels get permuted by each
  conv's PSUM col-block assignment; the final permutation is undone on
  the host.
- All matmul operands (weights, spikes, conv1 im2col) are fp16;
  accumulation stays fp32 in PSUM.
- Convs: 9 shifted matmuls accumulating in PSUM over padded SBUF spike
  slots. Small-C layers use TensorE sub-array tiling (tile_position),
  up to 16 concurrent (K=32,M=32) tiles.
- conv1 exploits the T-broadcast of the input: computed once (im2col
  K=27).
- All pre-BN conv outputs (pb buffers) live in SBUF as fp16 — no DRAM
  round trip. Eviction is an ACT Copy (PSUM->pb fp16, accum_out =
  per-channel sums) plus an ACT Square (PSUM->scratch, accum_out =
  sumsq).
- LIF(L) and conv(L+1) are interleaved per timestep through
  double-buffered spike slots, so VectorE (LIF) overlaps TensorE
  (conv) across the layer boundary.
- LIF runs in "p-space" (p_t = v_t * 2^t), all fp16 on VectorE using
  only standard DVE ops (they hit the packed 2x/4x modes; the custom
  DVE ops and STT run at 1x and are avoided):
    tmp   = x_t*(inv*2^(t-1)) + shift*2^(t-1)   (tensor_scalar)
    p_t   = tmp + pk_{t-1}                      (tensor_tensor add)
    spike = p_t >= 2^t                          (tensor_scalar is_ge)
    nsp   = p_t < 2^t                           (tensor_scalar is_lt)
    pk_t  = p_t * nsp                           (tensor_tensor mult)
  MaxPool folds into the spike op (spike of max(p) over the 2x2
  window).
- Tiny "tickle" matmuls chained to LIF tiles keep the PE's HAM
  activity window busy so conv bursts run at the warm 2.4 GHz clock.
"""
import os
import numpy as np

import concourse.bass as bass
import concourse.tile as tile
import concourse.mybir as mybir
from concourse import bacc

F32 = mybir.dt.float32
F16 = mybir.dt.float16
Alu = mybir.AluOpType
Act = mybir.ActivationFunctionType

T = 8
N_CORES = 8
N_LOC = 16
EPS = 1e-5
DEBUG = bool(os.environ.get("SNN_DEBUG"))

LCFG = [
    dict(name='2', ci=32, co=32, h=32, pool=True),
    dict(name='3', ci=32, co=64, h=16, pool=False),
    dict(name='4', ci=64, co=64, h=16, pool=True),
    dict(name='5', ci=64, co=128, h=8, pool=False),
    dict(name='6', ci=128, co=128, h=8, pool=True),
]
for L in LCFG:
    L['gi'] = 128 // L['ci']
    L['si'] = N_LOC // L['gi']
    L['go'] = 128 // L['co']
    L['so_cnt'] = N_LOC // L['go']


def _slot_maps():
    cur = [[4 * q + g for q in range(4)] for g in range(4)]
    for L in LCFG:
        gi, si, go = L['gi'], L['si'], L['go']
        nxt = [[None] * (N_LOC // go) for _ in range(go)]
        for g in range(gi):
            for s in range(si):
                j = s % go
                so = g * (si // go) + s // go
                nxt[j][so] = cur[g][s]
        cur = nxt
    return cur[0]


FINAL_SLOTS = _slot_maps()


def build_module():
    nc = bacc.Bacc(trn_type="TRN2", num_devices=N_CORES, name="snn",
                   dynamic_dma_scratch_size=2048)

    D = {}
    D['xpad'] = nc.dram_tensor("xpad", [3, N_LOC, 34, 34], F16,
                               kind="ExternalInput").ap()
    D['w1'] = nc.dram_tensor("w1im", [9, 3 * 32], F16,
                             kind="ExternalInput").ap()
    D['wd'] = {}
    D['bn'] = {}
    for L in LCFG:
        s = L['name']
        D['wd'][s] = nc.dram_tensor(f"w{s}", [L['ci'], 9, L['co']], F16,
                                    kind="ExternalInput").ap()
    for s in ['1', '2', '3', '4', '5', '6']:
        D['bn'][s] = nc.dram_tensor(f"bn{s}", [128, 3], F32,
                                    kind="ExternalInput").ap()
    D['fc1w'] = nc.dram_tensor("fc1w", [128, 16, 128], F16,
                               kind="ExternalInput").ap()
    D['fc1b'] = nc.dram_tensor("fc1b", [128, 1], F32,
                               kind="ExternalInput").ap()
    D['fc2w'] = nc.dram_tensor("fc2w", [128, 10], F16,
                               kind="ExternalInput").ap()
    D['fc2b'] = nc.dram_tensor("fc2b", [10, 1], F32,
                               kind="ExternalInput").ap()
    D['out'] = nc.dram_tensor("out", [10, N_LOC], F32,
                              kind="ExternalOutput").ap()
    if DEBUG:
        D['o2dbg'] = nc.dram_tensor("o2dbg", [10, 128], F32,
                                    kind="ExternalOutput").ap()
        D['h1dbg'] = nc.dram_tensor("h1dbg", [128, 128], F32,
                                    kind="ExternalOutput").ap()
    # local-batch BN: stats over this core's 16-image shard only
    D['cnt'] = {'1': N_LOC * 1024.0, '2': 8 * N_LOC * 1024.0,
                '3': 8 * N_LOC * 256.0, '4': 8 * N_LOC * 256.0,
                '5': 8 * N_LOC * 64.0, '6': 8 * N_LOC * 64.0}

    from contextlib import ExitStack
    with tile.TileContext(nc) as tc:
        with ExitStack() as es:
            build_body(nc, tc, es, D)
    nc.compile()
    return nc


def build_body(nc, tc, es, D):
    glob = es.enter_context(tc.tile_pool(name="glob", bufs=1))
    ppool = es.enter_context(tc.tile_pool(name="ppool", bufs=2))
    mxp = es.enter_context(tc.tile_pool(name="mxp", bufs=1))
    sqp = es.enter_context(tc.tile_pool(name="sqp", bufs=1))
    spp = es.enter_context(tc.tile_pool(name="spp", bufs=2))
    pbp = es.enter_context(tc.tile_pool(name="pbp", bufs=1))
    psum = es.enter_context(tc.tile_pool(name="psum", bufs=4, space="PSUM"))

    AB = {}
    for s in ['1', '2', '3', '4', '5', '6']:
        AB[s] = (glob.tile([128, 8], F32, tag=f"A{s}", name=f"A{s}"),
                 glob.tile([128, 8], F32, tag=f"B{s}", name=f"B{s}"))
    pow2row = glob.tile([128, 8], F32, tag="pow2", name="pow2row")
    for t in range(T):
        nc.vector.memset(pow2row[:, t:t + 1], float(2.0 ** (t - 1)))

    # ---- preload all weights ----
    w1_sb = glob.tile([9, 3 * 32], F16, tag="w1", name="w1")
    nc.sync.dma_start(w1_sb[:], D['w1'][:])
    WS = {}
    for L in LCFG:
        s = L['name']
        ci, gi = L['ci'], L['gi']
        w_sb = glob.tile([128, 9 * L['co']], F16, tag=f"w{s}", name=f"w{s}")
        src = D['wd'][s][:].rearrange("ci k co -> ci (k co)")
        for g in range(gi):
            nc.sync.dma_start(w_sb[g * ci:(g + 1) * ci, :], src)
        WS[s] = w_sb
    fc1w = glob.tile([128, 16 * 128], F16, tag="fc1w", name="fc1w")
    nc.sync.dma_start(fc1w[:], D['fc1w'][:].rearrange("c s o -> c (s o)"))
    fc1b = glob.tile([128, 1], F32, tag="fc1b", name="fc1b")
    nc.sync.dma_start(fc1b[:], D['fc1b'][:])
    fc2w = glob.tile([128, 10], F16, tag="fc2w", name="fc2w")
    nc.sync.dma_start(fc2w[:], D['fc2w'][:])
    fc2b = glob.tile([10, 1], F32, tag="fc2b", name="fc2b")
    nc.sync.dma_start(fc2b[:], D['fc2b'][:])

    def evict(psrc, ddst, ssum_col, ssq_col):
        """ACT Copy psum->pb fp16 (+sum), ACT Square psum->scratch
        (+sumsq)."""
        npart = psrc.shape[0]
        fd = psrc.free_size()
        sq = sqp.tile([128, 1024], F32, tag="sq", name="sq")
        nc.scalar.activation(ddst, psrc, Act.Copy, accum_out=ssum_col)
        nc.scalar.activation(sq[0:npart, 0:fd], psrc, Act.Square,
                             accum_out=ssq_col)

    def finalize_bn(s, ssum_strip, ssq_strip, go, co):
        bnp = glob.tile([128, 3], F32, tag=f"bn{s}", name=f"bnp{s}")
        nc.sync.dma_start(bnp[:], D['bn'][s][:])
        tot = glob.tile([128, 2], F32, tag=f"tot{s}", name=f"tot{s}")
        nc.vector.reduce_sum(tot[:, 0:1], ssum_strip[:],
                             axis=mybir.AxisListType.X)
        nc.vector.reduce_sum(tot[:, 1:2], ssq_strip[:],
                             axis=mybir.AxisListType.X)
        if go > 1:
            # cross-partition-base TT is illegal: stage the blocks into
            # base-aligned columns, add columns, then broadcast back.
            fold = glob.tile([128, 2 * 4], F32, tag=f"fold{s}",
                             name=f"fold{s}")
            for g in range(1, go):
                nc.vector.tensor_copy(fold[0:co, 2 * g:2 * g + 2],
                                      tot[g * co:(g + 1) * co, :])
            for g in range(1, go):
                nc.vector.tensor_tensor(tot[0:co, :], tot[0:co, :],
                                        fold[0:co, 2 * g:2 * g + 2],
                                        Alu.add)
            for g in range(1, go):
                nc.vector.tensor_copy(tot[g * co:(g + 1) * co, :],
                                      tot[0:co, :])
        sc = glob.tile([128, 6], F32, tag=f"sc{s}", name=f"sc{s}")
        m, ex2, var, inv, sh, tmp = [sc[:, i:i + 1] for i in range(6)]
        icnt = 1.0 / D['cnt'][s]
        nc.vector.tensor_scalar(m, tot[:, 0:1], icnt, None, Alu.mult)
        nc.vector.tensor_scalar(ex2, tot[:, 1:2], icnt, None, Alu.mult)
        nc.vector.tensor_tensor(tmp, m, m, Alu.mult)
        nc.vector.tensor_tensor(var, ex2, tmp, Alu.subtract)
        nc.vector.tensor_scalar(var, var, EPS, None, Alu.add)
        nc.scalar.activation(tmp, var, Act.Sqrt)
        nc.vector.reciprocal(var, tmp)
        nc.vector.tensor_tensor(inv, var, bnp[:, 0:1], Alu.mult)
        nc.vector.tensor_tensor(sh, bnp[:, 2:3], m, Alu.subtract)
        nc.vector.tensor_tensor(sh, sh, inv, Alu.mult)
        nc.vector.tensor_tensor(sh, sh, bnp[:, 1:2], Alu.add)
        A, B = AB[s]
        nc.vector.tensor_scalar(A[:], pow2row[:], inv, None, Alu.mult)
        nc.vector.tensor_scalar(B[:], pow2row[:], sh, None, Alu.mult)

    def sp_slots(L_next, padded=True):
        """Two rotating per-t spike slot tiles, halos pre-zeroed."""
        h = L_next['h']
        hp = h + 2 if padded else h
        si = L_next['si']
        slots = []
        for b in range(2):
            tl = spp.tile([128, si, hp, hp], F16, tag=f"sp{L_next['name']}",
                          name=f"sp{L_next['name']}_{b}")
            if padded:
                nc.vector.memset(tl[:, :, 0:1, :], 0.0)
                nc.vector.memset(tl[:, :, hp - 1:hp, :], 0.0)
                nc.vector.memset(tl[:, :, :, 0:1], 0.0)
                nc.vector.memset(tl[:, :, :, hp - 1:hp], 0.0)
            slots.append(tl)
        return slots

    last_pst = [None]

    def pe_tickle(src_tile):
        """Tiny matmul chained to src_tile, accumulating garbage into
        the previous (already-evicted, about-to-be-recycled) PSUM tile:
        keeps the PE HAM activity window busy during vector-dominated
        stretches so conv bursts run at the warm 2.4 GHz clock instead
        of re-throttled 1.2. The target region is never read before
        its next start=True clear, so the garbage is inert."""
        if last_pst[0] is None:
            return
        nc.tensor.matmul(last_pst[0][0:32, 0:4], w1_sb[:, 0:32],
                         src_tile[0:9, 0:4], start=False, stop=False,
                         skip_group_check=True)

    def lif_step(s_lif, t, xin, fd, pk, dst, pool_shape):
        """One LIF timestep: returns new pk tile (or None at t=T-1).
        xin: [128, fd] AP of pre-BN x_t. dst: spike destination AP
        (padded interior view already sliced). pool_shape: None or
        (so, h) to maxpool p before thresholding. The recurrence runs
        on VectorE with standard fp16 ops; spike generation (and
        pooling) is offloaded to GpSimd for fd >= 2048."""
        A, B = AB[s_lif]
        th = float(2.0 ** t)
        p = ppool.tile([128, fd], F16, tag="p", bufs=3, name="p")
        if t == 0:
            nc.vector.tensor_scalar(p[:], xin, A[:, 0:1], B[:, 0:1],
                                    Alu.mult, Alu.add)
        else:
            tmp = ppool.tile([128, fd], F16, tag="p", bufs=3, name="tmp")
            nc.vector.tensor_scalar(tmp[:], xin, A[:, t:t + 1],
                                    B[:, t:t + 1], Alu.mult, Alu.add)
            nc.vector.tensor_tensor(p[:], tmp[:], pk[:], Alu.add)
        pe_tickle(p)
        if pool_shape is not None:
            so, h = pool_shape
            pv = p[:].rearrange("c (so y x) -> c so y x", so=so, y=h, x=h)
            mx = mxp.tile([128, so * h * (h // 2)], F16, tag="mx", name="mx")
            mxv = mx[:].rearrange("c (so y x) -> c so y x",
                                  so=so, y=h, x=h // 2)
            nc.vector.tensor_tensor(mxv[:], pv[:, :, :, 0:h:2],
                                    pv[:, :, :, 1:h:2], Alu.max)
            myv = mxv[:, :, 0:h:2, :]
            nc.vector.tensor_tensor(myv, mxv[:, :, 0:h:2, :],
                                    mxv[:, :, 1:h:2, :], Alu.max)
            src = myv
        else:
            src = p[:]
        nc.vector.tensor_scalar(dst, src, th, None, Alu.is_ge)
        if t < T - 1:
            nsp = ppool.tile([128, fd], F16, tag="p", bufs=3, name="nsp")
            nc.vector.tensor_scalar(nsp[:], p[:], th, None, Alu.is_lt)
            pk2 = ppool.tile([128, fd], F16, tag="p", bufs=3, name="pk")
            nc.vector.tensor_tensor(pk2[:], p[:], nsp[:], Alu.mult)
            pe_tickle(pk2)
            return pk2
        return None

    def conv_t(L, sp_t, w_sb, t, pb_tile, ssum, ssq, ecol):
        """One timestep of conv L: matmuls from spike slot sp_t into
        PSUM, evict into pb_tile[:, t, ...] (+stats cols from ecol)."""
        s = L['name']
        ci, co, gi, go, h = L['ci'], L['co'], L['gi'], L['go'], L['h']
        hw = h * h
        ipc = max(1, 512 // hw)

        def one_mm(g, j, chunk, k, out_sl, start, stop):
            dy, dx = k // 3, k % 3
            if ipc == 1:
                nr = 512 // h
                r0 = chunk * nr
                rhs = sp_t[ci * g:ci * g + ci, j,
                           r0 + dy:r0 + dy + nr, dx:dx + h]
            else:
                s0 = j + go * chunk * ipc
                rhs = sp_t[ci * g:ci * g + ci,
                           s0:s0 + go * (ipc - 1) + 1:go,
                           dy:dy + h, dx:dx + h]
            tp = None
            if ci < 128 or co < 128:
                tp = (ci * g, co * j)
            nc.tensor.matmul(
                out_sl, w_sb[ci * g:ci * g + ci, co * k:co * k + co],
                rhs, start=start, stop=stop, tile_position=tp,
                skip_group_check=True)

        def do_evict(dst_flat, pslice):
            evict(pslice, dst_flat,
                  ssum[:, ecol[0]:ecol[0] + 1],
                  ssq[:, ecol[0]:ecol[0] + 1])
            ecol[0] += 1

        if gi == 4 and go == 4:           # L2: 16 tiles (4g x 4j), 2hh
            psts = [psum.tile([128, 1024], F32, tag="ps", name="ps")
                    for _ in range(4)]
            for k in range(9):
                dy, dx = k // 3, k % 3
                for hh in range(2):
                    for g in range(4):
                        for j in range(4):
                            rhs = sp_t[32 * g:32 * g + 32, j,
                                       16 * hh + dy:16 * hh + dy + 16,
                                       dx:dx + 32]
                            nc.tensor.matmul(
                                psts[g][32 * j:32 * j + 32,
                                        512 * hh:512 * hh + 512],
                                w_sb[32 * g:32 * g + 32,
                                     32 * k:32 * k + 32],
                                rhs, start=(k == 0), stop=(k == 8),
                                tile_position=(32 * g, 32 * j),
                                skip_group_check=True)
            for g in range(4):
                do_evict(pb_tile[:, t, g].rearrange("c y x -> c (y x)"),
                         psts[g][:])
            last_pst[0] = psts[3]
        elif gi == 1:                     # L6: one tile, 2 chunks
            pst = psum.tile([128, 1024], F32, tag="ps", name="ps")
            for k in range(9):
                for chunk in range(2):
                    one_mm(0, 0, chunk, k,
                           pst[:, 512 * chunk:512 * chunk + 512],
                           k == 0, k == 8)
            do_evict(pb_tile[:, t].rearrange("c s y x -> c (s y x)"),
                     pst[:])
            last_pst[0] = pst
        elif go == 1:                     # L5: 2 row tiles
            pst = psum.tile([128, 1024], F32, tag="ps", name="ps")
            for k in range(9):
                for g in range(gi):
                    one_mm(g, 0, 0, k,
                           pst[:, 512 * g:512 * g + 512],
                           k == 0, k == 8)
            do_evict(pb_tile[:, t].rearrange("c s y x -> c (s y x)"),
                     pst[:])
            last_pst[0] = pst
        elif ci == 32:                    # L3: 8 tiles (2q x 2u x 2j)
            psts = [psum.tile([128, 1024], F32, tag="ps", name="ps")
                    for _ in range(2)]
            for k in range(9):
                for q in range(2):
                    for u in range(2):
                        for j in range(go):
                            one_mm(2 * q + u, j, 0, k,
                                   psts[q][64 * j:64 * j + 64,
                                           512 * u:512 * u + 512],
                                   k == 0, k == 8)
            for q in range(2):
                do_evict(
                    pb_tile[:, t, 4 * q:4 * q + 4].rearrange(
                        "c s y x -> c (s y x)"),
                    psts[q][:])
            last_pst[0] = psts[1]
        else:                             # L4: 4 tiles (2g x 2j), 2v
            psts = [psum.tile([128, 1024], F32, tag="ps", name="ps")
                    for _ in range(2)]
            for k in range(9):
                for v in range(2):
                    for g in range(gi):
                        for j in range(go):
                            one_mm(g, j, v, k,
                                   psts[g][64 * j:64 * j + 64,
                                           512 * v:512 * v + 512],
                                   k == 0, k == 8)
            for g in range(2):
                do_evict(
                    pb_tile[:, t, 4 * g:4 * g + 4].rearrange(
                        "c s y x -> c (s y x)"),
                    psts[g][:])
            last_pst[0] = psts[1]

    # ================= Stage 1: conv1 + BN1 =================
    y1 = glob.tile([128, 4, 32, 32], F16, tag="y1", name="y1")
    ssum1 = glob.tile([128, 4], F32, tag="ssum1", name="ssum1")
    ssq1 = glob.tile([128, 4], F32, tag="ssq1", name="ssq1")
    nc.vector.memset(ssum1[:], 0.0)
    nc.vector.memset(ssq1[:], 0.0)

    # dy-stacked input: partition block d holds xpad shifted down by d,
    # so one K=9 matmul contracts over (channel, dy); dx becomes 3
    # accumulation steps via free-dim shifted views.
    xpad = D['xpad']
    xs3 = pbp.tile([9, N_LOC, 32, 34], F16, tag="pbA", name="xs3")
    for dy in range(3):
        nc.sync.dma_start(xs3[3 * dy:3 * dy + 3, :, :, :],
                          xpad[:, :, dy:dy + 32, :])
    for q in range(4):
        pst = psum.tile([128, 1024], F32, tag="ps", name="ps")
        for dx in range(3):
            for hh in range(2):
                for r in range(4):
                    nc.tensor.matmul(
                        pst[32 * r:32 * r + 32, 512 * hh:512 * hh + 512],
                        w1_sb[:, 32 * dx:32 * dx + 32],
                        xs3[:, 4 * q + r, 16 * hh:16 * hh + 16,
                            dx:dx + 32],
                        start=(dx == 0), stop=(dx == 2),
                        tile_position=(0, 32 * r), skip_group_check=True)
        sq = sqp.tile([128, 1024], F32, tag="sq", name="sq")
        nc.scalar.activation(
            y1[:, q, :, :].rearrange("c y x -> c (y x)"),
            pst[:], Act.Copy, accum_out=ssum1[:, q:q + 1])
        nc.scalar.activation(sq[:], pst[:], Act.Square,
                             accum_out=ssq1[:, q:q + 1])
        last_pst[0] = pst
    finalize_bn('1', ssum1, ssq1, 4, 32)

    # ============ Stage 2: LIF1 + conv2 (per-t interleaved) ============
    l2 = LCFG[0]
    ssum2 = glob.tile([128, 32], F32, tag="ssum2", name="ssum2")
    ssq2 = glob.tile([128, 32], F32, tag="ssq2", name="ssq2")
    nc.vector.memset(ssum2[:], 0.0)
    nc.vector.memset(ssq2[:], 0.0)
    pb2 = pbp.tile([128, T, 4, 32, 32], F16, tag="pbA", name="pb2")

    stg_slots = sp_slots(l2, padded=True)
    y1flat = y1[:].rearrange("c s y x -> c (s y x)")
    pk = None
    ecol2 = [0]
    for t in range(T):
        slot = stg_slots[t % 2]
        pk = lif_step('1', t, y1flat, 4096, pk,
                      slot[:, :, 1:33, 1:33], None)
        conv_t(l2, slot, WS['2'], t, pb2, ssum2, ssq2, ecol2)
    finalize_bn('2', ssum2, ssq2, 4, 32)

    # ============ Chain: LIF(prev) + conv(L) per-t ============
    N_EV = {'3': 16, '4': 16, '5': 8, '6': 8}
    PB_TAG = {'3': 'pbB', '4': 'pbA', '5': 'pbB', '6': 'pbA'}
    prev_L = l2
    prev_pb = pb2
    for idx in range(1, len(LCFG)):
        nxt = LCFG[idx]
        sn, sp_ = nxt['name'], prev_L['name']
        slots = sp_slots(nxt, padded=True)
        ssum_n = glob.tile([128, N_EV[sn]], F32, tag=f"ssum{sn}",
                           name=f"ssum{sn}")
        ssq_n = glob.tile([128, N_EV[sn]], F32, tag=f"ssq{sn}",
                          name=f"ssq{sn}")
        nc.vector.memset(ssum_n[:], 0.0)
        nc.vector.memset(ssq_n[:], 0.0)
        pb_n = pbp.tile([128, T, nxt['so_cnt'], nxt['h'], nxt['h']], F16,
                        tag=PB_TAG[sn], name=f"pb{sn}")

        so_p, h_p = prev_L['so_cnt'], prev_L['h']
        fd_p = so_p * h_p * h_p
        ho = h_p // 2 if prev_L['pool'] else h_p
        pool_shape = (so_p, h_p) if prev_L['pool'] else None
        pk = None
        ecol = [0]
        for t in range(T):
            slot = slots[t % 2]
            xin = prev_pb[:, t].rearrange("c s y x -> c (s y x)")
            pk = lif_step(sp_, t, xin, fd_p, pk,
                          slot[:, :, 1:ho + 1, 1:ho + 1], pool_shape)
            conv_t(nxt, slot, WS[sn], t, pb_n, ssum_n, ssq_n, ecol)
        finalize_bn(sn, ssum_n, ssq_n, nxt['go'], nxt['co'])
        prev_L = nxt
        prev_pb = pb_n

    # ===== LIF6 -> s6, fc1 matmuls interleaved per-t =====
    bf1 = glob.tile([128, 8], F32, tag="bf1", name="bf1")
    bf2 = glob.tile([10, 8], F32, tag="bf2", name="bf2")
    nc.vector.tensor_scalar(bf1[:], pow2row[:], fc1b[:, 0:1], None, Alu.mult)
    nc.vector.tensor_scalar(bf2[:], pow2row[0:10, :], fc2b[:, 0:1], None,
                            Alu.mult)

    s6 = glob.tile([128, T, 16, 4, 4], F16, tag="s6", name="s6")
    s6v = s6[:].rearrange("c t s y x -> c t s (y x)")
    pstf = psum.tile([128, 1024], F32, tag="ps", name="psfc")
    pfc = pstf[:, 0:128]
    pk = None
    for t in range(T):
        xin = prev_pb[:, t].rearrange("c s y x -> c (s y x)")
        pk = lif_step('6', t, xin, 1024, pk,
                      s6[:, t].rearrange("c s y x -> c (s y x)"),
                      (16, 8))
        for pos in range(16):
            nc.tensor.matmul(pfc[:, 16 * t:16 * t + 16],
                             fc1w[:, pos * 128:(pos + 1) * 128],
                             s6v[:, t, :, pos],
                             start=(pos == 0), stop=(pos == 15),
                             skip_group_check=True)
    if DEBUG:
        h1 = glob.tile([128, 128], F32, tag="h1", name="h1")
        nc.scalar.activation(h1[:], pfc, Act.Copy)
        nc.sync.dma_start(D['h1dbg'], h1[:])

    # ===== fc1 LIF (reads PSUM directly) =====
    h1s = glob.tile([128, 128], F16, tag="h1s", name="h1s")
    pk = None
    for t in range(T):
        th = float(2.0 ** t)
        p = ppool.tile([128, 16], F32, tag="pf", bufs=3, name="pf")
        xin = pfc[:, 16 * t:16 * t + 16]
        if t == 0:
            nc.vector.tensor_scalar(p[:], xin, 0.5, bf1[:, 0:1],
                                    Alu.mult, Alu.add)
        else:
            tmp = ppool.tile([128, 16], F32, tag="pf", bufs=3, name="pft")
            nc.vector.tensor_scalar(tmp[:], xin, float(2.0 ** (t - 1)),
                                    bf1[:, t:t + 1], Alu.mult, Alu.add)
            nc.vector.tensor_tensor(p[:], tmp[:], pk[:], Alu.add)
        nc.vector.tensor_scalar(h1s[:, 16 * t:16 * t + 16], p[:], th, None,
                                Alu.is_ge)
        if t < T - 1:
            pk2 = ppool.tile([128, 16], F32, tag="pf", bufs=3, name="pfk")
            nc.vector.scalar_tensor_tensor(pk2[:], p[:], th, p[:],
                                           Alu.is_lt, Alu.mult)
            pk = pk2

    pst2 = psum.tile([128, 1024], F32, tag="ps", name="ps2")
    po = pst2[0:10, 0:128]
    nc.tensor.matmul(po, fc2w[:], h1s[:], start=True, stop=True)
    if DEBUG:
        o2d = glob.tile([10, 128], F32, tag="o2", name="o2")
        nc.scalar.activation(o2d[:], po, Act.Copy)
        nc.sync.dma_start(D['o2dbg'], o2d[:])

    # ===== fc2 LIF + rate accumulation (reads PSUM directly) =====
    oacc = glob.tile([10, 16], F32, tag="oaccA", name="oacc")
    pk = None
    for t in range(T):
        th = float(2.0 ** t)
        p = ppool.tile([10, 16], F32, tag="pf", bufs=3, name="pg")
        xin = po[:, 16 * t:16 * t + 16]
        if t == 0:
            nc.vector.tensor_scalar(p[:], xin, 0.5, bf2[:, 0:1],
                                    Alu.mult, Alu.add)
        else:
            tmp = ppool.tile([10, 16], F32, tag="pf", bufs=3, name="pgt")
            nc.vector.tensor_scalar(tmp[:], xin, float(2.0 ** (t - 1)),
                                    bf2[:, t:t + 1], Alu.mult, Alu.add)
            nc.vector.tensor_tensor(p[:], tmp[:], pk[:], Alu.add)
        spk = glob.tile([10, 16], F32, tag=f"spk{t % 2}", name="spk")
        nc.vector.tensor_scalar(spk[:], p[:], th, None, Alu.is_ge)
        if t == 0:
            nc.vector.tensor_scalar(oacc[:], spk[:], 1.0 / T, None, Alu.mult)
        else:
            oacc2 = glob.tile([10, 16], F32, tag=f"oacc{t % 2}",
                              name="oacc2")
            nc.vector.scalar_tensor_tensor(oacc2[:], spk[:], 1.0 / T,
                                           oacc[:], Alu.mult, Alu.add)
            oacc = oacc2
        if t < T - 1:
            pk2 = ppool.tile([10, 16], F32, tag="pf", bufs=3, name="pgk")
            nc.vector.scalar_tensor_tensor(pk2[:], p[:], th, p[:],
                                           Alu.is_lt, Alu.mult)
            pk = pk2

    nc.sync.dma_start(D['out'], oacc[:])


# ===================== host side =====================
_CACHE = {}


def _get_module():
    if "nc" not in _CACHE:
        _CACHE["nc"] = build_module()
    return _CACHE["nc"]


def _prep_inputs(inputs):
    x = np.ascontiguousarray(np.asarray(inputs['x'], np.float32))
    N = x.shape[0]
    n_loc = N // N_CORES

    w1 = np.asarray(inputs['w1'], np.float32)
    w1im = np.zeros((9, 3 * 32), np.float32)
    for dy in range(3):
        for dx in range(3):
            for c in range(3):
                w1im[dy * 3 + c, 32 * dx:32 * dx + 32] = w1[:, c, dy, dx]

    shared = {"w1im": w1im.astype(np.float16)}
    for L in LCFG:
        s = L['name']
        w = np.asarray(inputs['w' + s], np.float32)
        shared[f"w{s}"] = np.ascontiguousarray(
            w.transpose(1, 2, 3, 0).reshape(L['ci'], 9, L['co'])
        ).astype(np.float16)
    for s, go in [('1', 4), ('2', 4), ('3', 2), ('4', 2), ('5', 1),
                  ('6', 1)]:
        g = np.tile(np.asarray(inputs['g' + s], np.float32), go)
        be = np.tile(np.asarray(inputs['be' + s], np.float32), go)
        b = np.tile(np.asarray(inputs['b' + s], np.float32), go)
        shared[f"bn{s}"] = np.ascontiguousarray(np.stack([g, be, b], axis=1))
    fc1w = np.asarray(inputs['fc1_w'], np.float32)
    shared["fc1w"] = np.ascontiguousarray(
        fc1w.reshape(128, 128, 16).transpose(1, 2, 0)).astype(np.float16)
    shared["fc1b"] = np.asarray(inputs['fc1_b'], np.float32).reshape(128, 1)
    shared["fc2w"] = np.ascontiguousarray(
        np.asarray(inputs['fc2_w'], np.float32).T).astype(np.float16)
    shared["fc2b"] = np.asarray(inputs['fc2_b'], np.float32).reshape(10, 1)

    in_maps = []
    for c in range(N_CORES):
        xs = x[c * n_loc:(c + 1) * n_loc]
        xp = np.zeros((3, n_loc, 34, 34), np.float16)
        xp[:, :, 1:33, 1:33] = xs.transpose(1, 0, 2, 3).astype(np.float16)
        m = dict(shared)
        m["xpad"] = np.ascontiguousarray(xp)
        in_maps.append(m)
    return in_maps


def kernel(**inputs) -> np.ndarray:
    from concourse.bass_utils import run_bass_kernel_spmd
    nc = _get_module()
    in_maps = _prep_inputs(inputs)
    res = run_bass_kernel_spmd(nc, in_maps, core_ids=list(range(N_CORES)))
    N = np.asarray(inputs['x']).shape[0]
    n_loc = N // N_CORES
    out = np.zeros((N, 10), np.float32)
    for c in range(N_CORES):
        o = res.results[c]["out"]
        for s_idx in range(n_loc):
            out[c * n_loc + FINAL_SLOTS[s_idx], :] = o[:, s_idx]
    return out


if __name__ == "__main__":
    _get_module()
    print("module built OK")


# revision 58
# speedup vs baseline: 1.1248x; 1.1054x over previous
"""Trainium2 Bass kernel for nn_EnhancedSNNCifar (8-core data parallel).

Strategy
--------
Pure data parallel: batch 128 -> 16 images per NeuronCore, all weights
replicated. BN uses local-batch statistics (per-shard, as sanctioned by
the sharding spec) so no collectives are needed.

Per-core kernel:
- Channels on partitions; when C < 128, image-groups are packed into
  the spare partition blocks. Group/slot labels get permuted by each
  conv's PSUM col-block assignment; the final permutation is undone on
  the host.
- All matmul operands (weights, spikes, conv1 im2col) are fp16;
  accumulation stays fp32 in PSUM.
- Convs: 9 shifted matmuls accumulating in PSUM over padded SBUF spike
  slots. Small-C layers use TensorE sub-array tiling (tile_position),
  up to 16 concurrent (K=32,M=32) tiles.
- conv1 exploits the T-broadcast of the input: computed once (im2col
  K=27).
- All pre-BN conv outputs (pb buffers) live in SBUF as fp16 — no DRAM
  round trip. Eviction is an ACT Copy (PSUM->pb fp16, accum_out =
  per-channel sums) plus an ACT Square (PSUM->scratch, accum_out =
  sumsq).
- LIF(L) and conv(L+1) are interleaved per timestep through
  double-buffered spike slots, so VectorE (LIF) overlaps TensorE
  (conv) across the layer boundary.
- LIF runs in "p-space" (p_t = v_t * 2^t), all fp16 on VectorE using
  only standard DVE ops (these hit the packed 2x/4x modes; custom DVE
  ops and scalar_tensor_tensor run at 1x and are avoided):
    tmp   = x_t*(inv*2^(t-1)) + shift*2^(t-1)   (tensor_scalar)
    p_t   = tmp + pk_{t-1}                      (tensor_tensor add)
    spike = p_t >= 2^t                          (tensor_scalar is_ge)
    nsp   = p_t < 2^t                           (tensor_scalar is_lt)
    pk_t  = p_t * nsp                           (tensor_tensor mult)
  MaxPool folds into the spike path (spike of max(p) over the 2x2
  window) before thresholding.
"""
import os
import numpy as np

import concourse.bass as bass
import concourse.tile as tile
import concourse.mybir as mybir
from concourse import bacc

F32 = mybir.dt.float32
F16 = mybir.dt.float16
Alu = mybir.AluOpType
Act = mybir.ActivationFunctionType

T = 8
N_CORES = 8
N_LOC = 16
EPS = 1e-5
DEBUG = bool(os.environ.get("SNN_DEBUG"))

LCFG = [
    dict(name='2', ci=32, co=32, h=32, pool=True),
    dict(name='3', ci=32, co=64, h=16, pool=False),
    dict(name='4', ci=64, co=64, h=16, pool=True),
    dict(name='5', ci=64, co=128, h=8, pool=False),
    dict(name='6', ci=128, co=128, h=8, pool=True),
]
for L in LCFG:
    L['gi'] = 128 // L['ci']
    L['si'] = N_LOC // L['gi']
    L['go'] = 128 // L['co']
    L['so_cnt'] = N_LOC // L['go']


def _slot_maps():
    cur = [[4 * q + g for q in range(4)] for g in range(4)]
    for L in LCFG:
        gi, si, go = L['gi'], L['si'], L['go']
        nxt = [[None] * (N_LOC // go) for _ in range(go)]
        for g in range(gi):
            for s in range(si):
                j = s % go
                so = g * (si // go) + s // go
                nxt[j][so] = cur[g][s]
        cur = nxt
    return cur[0]


FINAL_SLOTS = _slot_maps()


def build_module():
    nc = bacc.Bacc(trn_type="TRN2", num_devices=N_CORES, name="snn",
                   dynamic_dma_scratch_size=2048)

    D = {}
    D['xpad'] = nc.dram_tensor("xpad", [3, N_LOC, 34, 34], F16,
                               kind="ExternalInput").ap()
    D['w1'] = nc.dram_tensor("w1im", [9, 3 * 32], F16,
                             kind="ExternalInput").ap()
    D['wd'] = {}
    D['bn'] = {}
    for L in LCFG:
        s = L['name']
        D['wd'][s] = nc.dram_tensor(f"w{s}", [L['ci'], 9, L['co']], F16,
                                    kind="ExternalInput").ap()
    for s in ['1', '2', '3', '4', '5', '6']:
        D['bn'][s] = nc.dram_tensor(f"bn{s}", [128, 3], F32,
                                    kind="ExternalInput").ap()
    D['fc1w'] = nc.dram_tensor("fc1w", [128, 16, 128], F16,
                               kind="ExternalInput").ap()
    D['fc1b'] = nc.dram_tensor("fc1b", [128, 1], F32,
                               kind="ExternalInput").ap()
    D['fc2w'] = nc.dram_tensor("fc2w", [128, 10], F16,
                               kind="ExternalInput").ap()
    D['fc2b'] = nc.dram_tensor("fc2b", [10, 1], F32,
                               kind="ExternalInput").ap()
    D['out'] = nc.dram_tensor("out", [10, N_LOC], F32,
                              kind="ExternalOutput").ap()
    if DEBUG:
        D['o2dbg'] = nc.dram_tensor("o2dbg", [10, 128], F32,
                                    kind="ExternalOutput").ap()
        D['h1dbg'] = nc.dram_tensor("h1dbg", [128, 128], F32,
                                    kind="ExternalOutput").ap()
    # local-batch BN: stats over this core's 16-image shard only
    D['cnt'] = {'1': N_LOC * 1024.0, '2': 8 * N_LOC * 1024.0,
                '3': 8 * N_LOC * 256.0, '4': 8 * N_LOC * 256.0,
                '5': 8 * N_LOC * 64.0, '6': 8 * N_LOC * 64.0}

    from contextlib import ExitStack
    with tile.TileContext(nc) as tc:
        with ExitStack() as es:
            build_body(nc, tc, es, D)
    nc.compile()
    return nc


def build_body(nc, tc, es, D):
    glob = es.enter_context(tc.tile_pool(name="glob", bufs=1))
    ppool = es.enter_context(tc.tile_pool(name="ppool", bufs=2))
    mxp = es.enter_context(tc.tile_pool(name="mxp", bufs=1))
    sqp = es.enter_context(tc.tile_pool(name="sqp", bufs=1))
    spp = es.enter_context(tc.tile_pool(name="spp", bufs=2))
    pbp = es.enter_context(tc.tile_pool(name="pbp", bufs=1))
    psum = es.enter_context(tc.tile_pool(name="psum", bufs=4, space="PSUM"))

    AB = {}
    for s in ['1', '2', '3', '4', '5', '6']:
        AB[s] = (glob.tile([128, 8], F32, tag=f"A{s}", name=f"A{s}"),
                 glob.tile([128, 8], F32, tag=f"B{s}", name=f"B{s}"))
    pow2row = glob.tile([128, 8], F32, tag="pow2", name="pow2row")
    for t in range(T):
        nc.vector.memset(pow2row[:, t:t + 1], float(2.0 ** (t - 1)))

    # ---- preload all weights ----
    w1_sb = glob.tile([9, 3 * 32], F16, tag="w1", name="w1")
    nc.sync.dma_start(w1_sb[:], D['w1'][:])
    WS = {}
    for L in LCFG:
        s = L['name']
        ci, gi = L['ci'], L['gi']
        w_sb = glob.tile([128, 9 * L['co']], F16, tag=f"w{s}", name=f"w{s}")
        src = D['wd'][s][:].rearrange("ci k co -> ci (k co)")
        for g in range(gi):
            nc.sync.dma_start(w_sb[g * ci:(g + 1) * ci, :], src)
        WS[s] = w_sb
    fc1w = glob.tile([128, 16 * 128], F16, tag="fc1w", name="fc1w")
    nc.sync.dma_start(fc1w[:], D['fc1w'][:].rearrange("c s o -> c (s o)"))
    fc1b = glob.tile([128, 1], F32, tag="fc1b", name="fc1b")
    nc.sync.dma_start(fc1b[:], D['fc1b'][:])
    fc2w = glob.tile([128, 10], F16, tag="fc2w", name="fc2w")
    nc.sync.dma_start(fc2w[:], D['fc2w'][:])
    fc2b = glob.tile([10, 1], F32, tag="fc2b", name="fc2b")
    nc.sync.dma_start(fc2b[:], D['fc2b'][:])

    def evict(psrc, ddst, ssum_col, ssq_col):
        """ACT Copy psum->pb fp16 (+sum), ACT Square psum->scratch
        (+sumsq)."""
        npart = psrc.shape[0]
        fd = psrc.free_size()
        sq = sqp.tile([128, 1024], F32, tag="sq", name="sq")
        nc.scalar.activation(ddst, psrc, Act.Copy, accum_out=ssum_col)
        nc.scalar.activation(sq[0:npart, 0:fd], psrc, Act.Square,
                             accum_out=ssq_col)

    def finalize_bn(s, ssum_strip, ssq_strip, go, co):
        bnp = glob.tile([128, 3], F32, tag=f"bn{s}", name=f"bnp{s}")
        nc.sync.dma_start(bnp[:], D['bn'][s][:])
        tot = glob.tile([128, 2], F32, tag=f"tot{s}", name=f"tot{s}")
        nc.vector.reduce_sum(tot[:, 0:1], ssum_strip[:],
                             axis=mybir.AxisListType.X)
        nc.vector.reduce_sum(tot[:, 1:2], ssq_strip[:],
                             axis=mybir.AxisListType.X)
        if go > 1:
            # cross-partition-base TT is illegal: stage the blocks into
            # base-aligned columns, add columns, then broadcast back.
            fold = glob.tile([128, 2 * 4], F32, tag=f"fold{s}",
                             name=f"fold{s}")
            for g in range(1, go):
                nc.vector.tensor_copy(fold[0:co, 2 * g:2 * g + 2],
                                      tot[g * co:(g + 1) * co, :])
            for g in range(1, go):
                nc.vector.tensor_tensor(tot[0:co, :], tot[0:co, :],
                                        fold[0:co, 2 * g:2 * g + 2],
                                        Alu.add)
            for g in range(1, go):
                nc.vector.tensor_copy(tot[g * co:(g + 1) * co, :],
                                      tot[0:co, :])
        sc = glob.tile([128, 6], F32, tag=f"sc{s}", name=f"sc{s}")
        m, ex2, var, inv, sh, tmp = [sc[:, i:i + 1] for i in range(6)]
        icnt = 1.0 / D['cnt'][s]
        nc.vector.tensor_scalar(m, tot[:, 0:1], icnt, None, Alu.mult)
        nc.vector.tensor_scalar(ex2, tot[:, 1:2], icnt, None, Alu.mult)
        nc.vector.tensor_tensor(tmp, m, m, Alu.mult)
        nc.vector.tensor_tensor(var, ex2, tmp, Alu.subtract)
        nc.vector.tensor_scalar(var, var, EPS, None, Alu.add)
        nc.scalar.activation(tmp, var, Act.Sqrt)
        nc.vector.reciprocal(var, tmp)
        nc.vector.tensor_tensor(inv, var, bnp[:, 0:1], Alu.mult)
        nc.vector.tensor_tensor(sh, bnp[:, 2:3], m, Alu.subtract)
        nc.vector.tensor_tensor(sh, sh, inv, Alu.mult)
        nc.vector.tensor_tensor(sh, sh, bnp[:, 1:2], Alu.add)
        A, B = AB[s]
        nc.vector.tensor_scalar(A[:], pow2row[:], inv, None, Alu.mult)
        nc.vector.tensor_scalar(B[:], pow2row[:], sh, None, Alu.mult)

    def sp_slots(L_next, padded=True):
        """Two rotating per-t spike slot tiles, halos pre-zeroed."""
        h = L_next['h']
        hp = h + 2 if padded else h
        si = L_next['si']
        slots = []
        for b in range(2):
            tl = spp.tile([128, si, hp, hp], F16, tag=f"sp{L_next['name']}",
                          name=f"sp{L_next['name']}_{b}")
            if padded:
                nc.vector.memset(tl[:, :, 0:1, :], 0.0)
                nc.vector.memset(tl[:, :, hp - 1:hp, :], 0.0)
                nc.vector.memset(tl[:, :, :, 0:1], 0.0)
                nc.vector.memset(tl[:, :, :, hp - 1:hp], 0.0)
            slots.append(tl)
        return slots

    def lif_step(s_lif, t, xin, fd, pk, dst, pool_shape):
        """One LIF timestep: returns new pk tile (or None at t=T-1).
        xin: [128, fd] AP of pre-BN x_t. dst: spike destination AP
        (padded interior view already sliced). pool_shape: None or
        (so, h) to maxpool p before thresholding. The recurrence runs
        on VectorE with standard fp16 ops; spike generation (and
        pooling) is offloaded to GpSimd for fd >= 2048."""
        A, B = AB[s_lif]
        th = float(2.0 ** t)
        p = ppool.tile([128, fd], F16, tag="p", bufs=3, name="p")
        if t == 0:
            nc.vector.tensor_scalar(p[:], xin, A[:, 0:1], B[:, 0:1],
                                    Alu.mult, Alu.add)
        else:
            tmp = ppool.tile([128, fd], F16, tag="p", bufs=3, name="tmp")
            nc.vector.tensor_scalar(tmp[:], xin, A[:, t:t + 1],
                                    B[:, t:t + 1], Alu.mult, Alu.add)
            nc.vector.tensor_tensor(p[:], tmp[:], pk[:], Alu.add)
        if pool_shape is not None:
            so, h = pool_shape
            pv = p[:].rearrange("c (so y x) -> c so y x", so=so, y=h, x=h)
            mx = mxp.tile([128, so * h * (h // 2)], F16, tag="mx", name="mx")
            mxv = mx[:].rearrange("c (so y x) -> c so y x",
                                  so=so, y=h, x=h // 2)
            nc.vector.tensor_tensor(mxv[:], pv[:, :, :, 0:h:2],
                                    pv[:, :, :, 1:h:2], Alu.max)
            myv = mxv[:, :, 0:h:2, :]
            nc.vector.tensor_tensor(myv, mxv[:, :, 0:h:2, :],
                                    mxv[:, :, 1:h:2, :], Alu.max)
            src = myv
        else:
            src = p[:]
        nc.vector.tensor_scalar(dst, src, th, None, Alu.is_ge)
        if t < T - 1:
            nsp = ppool.tile([128, fd], F16, tag="p", bufs=3, name="nsp")
            nc.vector.tensor_scalar(nsp[:], p[:], th, None, Alu.is_lt)
            pk2 = ppool.tile([128, fd], F16, tag="p", bufs=3, name="pk")
            nc.vector.tensor_tensor(pk2[:], p[:], nsp[:], Alu.mult)
            return pk2
        return None

    def conv_t(L, sp_t, w_sb, t, pb_tile, ssum, ssq, ecol):
        """One timestep of conv L: matmuls from spike slot sp_t into
        PSUM, evict into pb_tile[:, t, ...] (+stats cols from ecol)."""
        s = L['name']
        ci, co, gi, go, h = L['ci'], L['co'], L['gi'], L['go'], L['h']
        hw = h * h
        ipc = max(1, 512 // hw)

        def one_mm(g, j, chunk, k, out_sl, start, stop):
            dy, dx = k // 3, k % 3
            if ipc == 1:
                nr = 512 // h
                r0 = chunk * nr
                rhs = sp_t[ci * g:ci * g + ci, j,
                           r0 + dy:r0 + dy + nr, dx:dx + h]
            else:
                s0 = j + go * chunk * ipc
                rhs = sp_t[ci * g:ci * g + ci,
                           s0:s0 + go * (ipc - 1) + 1:go,
                           dy:dy + h, dx:dx + h]
            tp = None
            if ci < 128 or co < 128:
                tp = (ci * g, co * j)
            nc.tensor.matmul(
                out_sl, w_sb[ci * g:ci * g + ci, co * k:co * k + co],
                rhs, start=start, stop=stop, tile_position=tp,
                skip_group_check=True)

        def do_evict(dst_flat, pslice):
            evict(pslice, dst_flat,
                  ssum[:, ecol[0]:ecol[0] + 1],
                  ssq[:, ecol[0]:ecol[0] + 1])
            ecol[0] += 1

        if gi == 4 and go == 4:           # L2: 16 tiles (4g x 4j), 2hh
            psts = [psum.tile([128, 1024], F32, tag="ps", name="ps")
                    for _ in range(4)]
            for k in range(9):
                dy, dx = k // 3, k % 3
                for hh in range(2):
                    for g in range(4):
                        for j in range(4):
                            rhs = sp_t[32 * g:32 * g + 32, j,
                                       16 * hh + dy:16 * hh + dy + 16,
                                       dx:dx + 32]
                            nc.tensor.matmul(
                                psts[g][32 * j:32 * j + 32,
                                        512 * hh:512 * hh + 512],
                                w_sb[32 * g:32 * g + 32,
                                     32 * k:32 * k + 32],
                                rhs, start=(k == 0), stop=(k == 8),
                                tile_position=(32 * g, 32 * j),
                                skip_group_check=True)
            for g in range(4):
                do_evict(pb_tile[:, t, g].rearrange("c y x -> c (y x)"),
                         psts[g][:])
        elif gi == 1:                     # L6: one tile, 2 chunks
            pst = psum.tile([128, 1024], F32, tag="ps", name="ps")
            for k in range(9):
                for chunk in range(2):
                    one_mm(0, 0, chunk, k,
                           pst[:, 512 * chunk:512 * chunk + 512],
                           k == 0, k == 8)
            do_evict(pb_tile[:, t].rearrange("c s y x -> c (s y x)"),
                     pst[:])
        elif go == 1:                     # L5: 2 row tiles
            pst = psum.tile([128, 1024], F32, tag="ps", name="ps")
            for k in range(9):
                for g in range(gi):
                    one_mm(g, 0, 0, k,
                           pst[:, 512 * g:512 * g + 512],
                           k == 0, k == 8)
            do_evict(pb_tile[:, t].rearrange("c s y x -> c (s y x)"),
                     pst[:])
        elif ci == 32:                    # L3: 8 tiles (2q x 2u x 2j)
            psts = [psum.tile([128, 1024], F32, tag="ps", name="ps")
                    for _ in range(2)]
            for k in range(9):
                for q in range(2):
                    for u in range(2):
                        for j in range(go):
                            one_mm(2 * q + u, j, 0, k,
                                   psts[q][64 * j:64 * j + 64,
                                           512 * u:512 * u + 512],
                                   k == 0, k == 8)
            for q in range(2):
                do_evict(
                    pb_tile[:, t, 4 * q:4 * q + 4].rearrange(
                        "c s y x -> c (s y x)"),
                    psts[q][:])
        else:                             # L4: 4 tiles (2g x 2j), 2v
            psts = [psum.tile([128, 1024], F32, tag="ps", name="ps")
                    for _ in range(2)]
            for k in range(9):
                for v in range(2):
                    for g in range(gi):
                        for j in range(go):
                            one_mm(g, j, v, k,
                                   psts[g][64 * j:64 * j + 64,
                                           512 * v:512 * v + 512],
                                   k == 0, k == 8)
            for g in range(2):
                do_evict(
                    pb_tile[:, t, 4 * g:4 * g + 4].rearrange(
                        "c s y x -> c (s y x)"),
                    psts[g][:])

    # ================= Stage 1: conv1 + BN1 =================
    y1 = glob.tile([128, 4, 32, 32], F16, tag="y1", name="y1")
    ssum1 = glob.tile([128, 4], F32, tag="ssum1", name="ssum1")
    ssq1 = glob.tile([128, 4], F32, tag="ssq1", name="ssq1")
    nc.vector.memset(ssum1[:], 0.0)
    nc.vector.memset(ssq1[:], 0.0)

    # dy-stacked input: partition block d holds xpad shifted down by d,
    # so one K=9 matmul contracts over (channel, dy); dx becomes 3
    # accumulation steps via free-dim shifted views.
    xpad = D['xpad']
    xs3 = pbp.tile([9, N_LOC, 32, 34], F16, tag="pbA", name="xs3")
    for dy in range(3):
        nc.sync.dma_start(xs3[3 * dy:3 * dy + 3, :, :, :],
                          xpad[:, :, dy:dy + 32, :])
    for q in range(4):
        pst = psum.tile([128, 1024], F32, tag="ps", name="ps")
        for dx in range(3):
            for hh in range(2):
                for r in range(4):
                    nc.tensor.matmul(
                        pst[32 * r:32 * r + 32, 512 * hh:512 * hh + 512],
                        w1_sb[:, 32 * dx:32 * dx + 32],
                        xs3[:, 4 * q + r, 16 * hh:16 * hh + 16,
                            dx:dx + 32],
                        start=(dx == 0), stop=(dx == 2),
                        tile_position=(0, 32 * r), skip_group_check=True)
        sq = sqp.tile([128, 1024], F32, tag="sq", name="sq")
        nc.scalar.activation(
            y1[:, q, :, :].rearrange("c y x -> c (y x)"),
            pst[:], Act.Copy, accum_out=ssum1[:, q:q + 1])
        nc.scalar.activation(sq[:], pst[:], Act.Square,
                             accum_out=ssq1[:, q:q + 1])
    finalize_bn('1', ssum1, ssq1, 4, 32)

    # ============ Stage 2: LIF1 + conv2 (per-t interleaved) ============
    l2 = LCFG[0]
    ssum2 = glob.tile([128, 32], F32, tag="ssum2", name="ssum2")
    ssq2 = glob.tile([128, 32], F32, tag="ssq2", name="ssq2")
    nc.vector.memset(ssum2[:], 0.0)
    nc.vector.memset(ssq2[:], 0.0)
    pb2 = pbp.tile([128, T, 4, 32, 32], F16, tag="pbA", name="pb2")

    stg_slots = sp_slots(l2, padded=True)
    y1flat = y1[:].rearrange("c s y x -> c (s y x)")
    pk = None
    ecol2 = [0]
    for t in range(T):
        slot = stg_slots[t % 2]
        pk = lif_step('1', t, y1flat, 4096, pk,
                      slot[:, :, 1:33, 1:33], None)
        conv_t(l2, slot, WS['2'], t, pb2, ssum2, ssq2, ecol2)
    finalize_bn('2', ssum2, ssq2, 4, 32)

    # ============ Chain: LIF(prev) + conv(L) per-t ============
    N_EV = {'3': 16, '4': 16, '5': 8, '6': 8}
    PB_TAG = {'3': 'pbB', '4': 'pbA', '5': 'pbB', '6': 'pbA'}
    prev_L = l2
    prev_pb = pb2
    for idx in range(1, len(LCFG)):
        nxt = LCFG[idx]
        sn, sp_ = nxt['name'], prev_L['name']
        slots = sp_slots(nxt, padded=True)
        ssum_n = glob.tile([128, N_EV[sn]], F32, tag=f"ssum{sn}",
                           name=f"ssum{sn}")
        ssq_n = glob.tile([128, N_EV[sn]], F32, tag=f"ssq{sn}",
                          name=f"ssq{sn}")
        nc.vector.memset(ssum_n[:], 0.0)
        nc.vector.memset(ssq_n[:], 0.0)
        pb_n = pbp.tile([128, T, nxt['so_cnt'], nxt['h'], nxt['h']], F16,
                        tag=PB_TAG[sn], name=f"pb{sn}")

        so_p, h_p = prev_L['so_cnt'], prev_L['h']
        fd_p = so_p * h_p * h_p
        ho = h_p // 2 if prev_L['pool'] else h_p
        pool_shape = (so_p, h_p) if prev_L['pool'] else None
        pk = None
        ecol = [0]
        for t in range(T):
            slot = slots[t % 2]
            xin = prev_pb[:, t].rearrange("c s y x -> c (s y x)")
            pk = lif_step(sp_, t, xin, fd_p, pk,
                          slot[:, :, 1:ho + 1, 1:ho + 1], pool_shape)
            conv_t(nxt, slot, WS[sn], t, pb_n, ssum_n, ssq_n, ecol)
        finalize_bn(sn, ssum_n, ssq_n, nxt['go'], nxt['co'])
        prev_L = nxt
        prev_pb = pb_n

    # ===== LIF6 -> s6, fc1 matmuls interleaved per-t =====
    bf1 = glob.tile([128, 8], F32, tag="bf1", name="bf1")
    bf2 = glob.tile([10, 8], F32, tag="bf2", name="bf2")
    nc.vector.tensor_scalar(bf1[:], pow2row[:], fc1b[:, 0:1], None, Alu.mult)
    nc.vector.tensor_scalar(bf2[:], pow2row[0:10, :], fc2b[:, 0:1], None,
                            Alu.mult)

    s6 = glob.tile([128, T, 16, 4, 4], F16, tag="s6", name="s6")
    s6v = s6[:].rearrange("c t s y x -> c t s (y x)")
    pstf = psum.tile([128, 1024], F32, tag="ps", name="psfc")
    pfc = pstf[:, 0:128]
    pk = None
    for t in range(T):
        xin = prev_pb[:, t].rearrange("c s y x -> c (s y x)")
        pk = lif_step('6', t, xin, 1024, pk,
                      s6[:, t].rearrange("c s y x -> c (s y x)"),
                      (16, 8))
        for pos in range(16):
            nc.tensor.matmul(pfc[:, 16 * t:16 * t + 16],
                             fc1w[:, pos * 128:(pos + 1) * 128],
                             s6v[:, t, :, pos],
                             start=(pos == 0), stop=(pos == 15),
                             skip_group_check=True)
    if DEBUG:
        h1 = glob.tile([128, 128], F32, tag="h1", name="h1")
        nc.scalar.activation(h1[:], pfc, Act.Copy)
        nc.sync.dma_start(D['h1dbg'], h1[:])

    # ===== fc1 LIF (reads PSUM directly) =====
    h1s = glob.tile([128, 128], F16, tag="h1s", name="h1s")
    pk = None
    for t in range(T):
        th = float(2.0 ** t)
        p = ppool.tile([128, 16], F32, tag="pf", bufs=3, name="pf")
        xin = pfc[:, 16 * t:16 * t + 16]
        if t == 0:
            nc.vector.tensor_scalar(p[:], xin, 0.5, bf1[:, 0:1],
                                    Alu.mult, Alu.add)
        else:
            tmp = ppool.tile([128, 16], F32, tag="pf", bufs=3, name="pft")
            nc.vector.tensor_scalar(tmp[:], xin, float(2.0 ** (t - 1)),
                                    bf1[:, t:t + 1], Alu.mult, Alu.add)
            nc.vector.tensor_tensor(p[:], tmp[:], pk[:], Alu.add)
        nc.vector.tensor_scalar(h1s[:, 16 * t:16 * t + 16], p[:], th, None,
                                Alu.is_ge)
        if t < T - 1:
            pk2 = ppool.tile([128, 16], F32, tag="pf", bufs=3, name="pfk")
            nc.vector.scalar_tensor_tensor(pk2[:], p[:], th, p[:],
                                           Alu.is_lt, Alu.mult)
            pk = pk2

    pst2 = psum.tile([128, 1024], F32, tag="ps", name="ps2")
    po = pst2[0:10, 0:128]
    nc.tensor.matmul(po, fc2w[:], h1s[:], start=True, stop=True)
    if DEBUG:
        o2d = glob.tile([10, 128], F32, tag="o2", name="o2")
        nc.scalar.activation(o2d[:], po, Act.Copy)
        nc.sync.dma_start(D['o2dbg'], o2d[:])

    # ===== fc2 LIF + rate accumulation (reads PSUM directly) =====
    oacc = glob.tile([10, 16], F32, tag="oaccA", name="oacc")
    pk = None
    for t in range(T):
        th = float(2.0 ** t)
        p = ppool.tile([10, 16], F32, tag="pf", bufs=3, name="pg")
        xin = po[:, 16 * t:16 * t + 16]
        if t == 0:
            nc.vector.tensor_scalar(p[:], xin, 0.5, bf2[:, 0:1],
                                    Alu.mult, Alu.add)
        else:
            tmp = ppool.tile([10, 16], F32, tag="pf", bufs=3, name="pgt")
            nc.vector.tensor_scalar(tmp[:], xin, float(2.0 ** (t - 1)),
                                    bf2[:, t:t + 1], Alu.mult, Alu.add)
            nc.vector.tensor_tensor(p[:], tmp[:], pk[:], Alu.add)
        spk = glob.tile([10, 16], F32, tag=f"spk{t % 2}", name="spk")
        nc.vector.tensor_scalar(spk[:], p[:], th, None, Alu.is_ge)
        if t == 0:
            nc.vector.tensor_scalar(oacc[:], spk[:], 1.0 / T, None, Alu.mult)
        else:
            oacc2 = glob.tile([10, 16], F32, tag=f"oacc{t % 2}",
                              name="oacc2")
            nc.vector.scalar_tensor_tensor(oacc2[:], spk[:], 1.0 / T,
                                           oacc[:], Alu.mult, Alu.add)
            oacc = oacc2
        if t < T - 1:
            pk2 = ppool.tile([10, 16], F32, tag="pf", bufs=3, name="pgk")
            nc.vector.scalar_tensor_tensor(pk2[:], p[:], th, p[:],
                                           Alu.is_lt, Alu.mult)
            pk = pk2

    nc.sync.dma_start(D['out'], oacc[:])


# ===================== host side =====================
_CACHE = {}


def _get_module():
    if "nc" not in _CACHE:
        _CACHE["nc"] = build_module()
    return _CACHE["nc"]


def _prep_inputs(inputs):
    x = np.ascontiguousarray(np.asarray(inputs['x'], np.float32))
    N = x.shape[0]
    n_loc = N // N_CORES

    w1 = np.asarray(inputs['w1'], np.float32)
    w1im = np.zeros((9, 3 * 32), np.float32)
    for dy in range(3):
        for dx in range(3):
            for c in range(3):
                w1im[dy * 3 + c, 32 * dx:32 * dx + 32] = w1[:, c, dy, dx]

    shared = {"w1im": w1im.astype(np.float16)}
    for L in LCFG:
        s = L['name']
        w = np.asarray(inputs['w' + s], np.float32)
        shared[f"w{s}"] = np.ascontiguousarray(
            w.transpose(1, 2, 3, 0).reshape(L['ci'], 9, L['co'])
        ).astype(np.float16)
    for s, go in [('1', 4), ('2', 4), ('3', 2), ('4', 2), ('5', 1),
                  ('6', 1)]:
        g = np.tile(np.asarray(inputs['g' + s], np.float32), go)
        be = np.tile(np.asarray(inputs['be' + s], np.float32), go)
        b = np.tile(np.asarray(inputs['b' + s], np.float32), go)
        shared[f"bn{s}"] = np.ascontiguousarray(np.stack([g, be, b], axis=1))
    fc1w = np.asarray(inputs['fc1_w'], np.float32)
    shared["fc1w"] = np.ascontiguousarray(
        fc1w.reshape(128, 128, 16).transpose(1, 2, 0)).astype(np.float16)
    shared["fc1b"] = np.asarray(inputs['fc1_b'], np.float32).reshape(128, 1)
    shared["fc2w"] = np.ascontiguousarray(
        np.asarray(inputs['fc2_w'], np.float32).T).astype(np.float16)
    shared["fc2b"] = np.asarray(inputs['fc2_b'], np.float32).reshape(10, 1)

    in_maps = []
    for c in range(N_CORES):
        xs = x[c * n_loc:(c + 1) * n_loc]
        xp = np.zeros((3, n_loc, 34, 34), np.float16)
        xp[:, :, 1:33, 1:33] = xs.transpose(1, 0, 2, 3).astype(np.float16)
        m = dict(shared)
        m["xpad"] = np.ascontiguousarray(xp)
        in_maps.append(m)
    return in_maps


def kernel(**inputs) -> np.ndarray:
    from concourse.bass_utils import run_bass_kernel_spmd
    nc = _get_module()
    in_maps = _prep_inputs(inputs)
    res = run_bass_kernel_spmd(nc, in_maps, core_ids=list(range(N_CORES)))
    N = np.asarray(inputs['x']).shape[0]
    n_loc = N // N_CORES
    out = np.zeros((N, 10), np.float32)
    for c in range(N_CORES):
        o = res.results[c]["out"]
        for s_idx in range(n_loc):
            out[c * n_loc + FINAL_SLOTS[s_idx], :] = o[:, s_idx]
    return out


if __name__ == "__main__":
    _get_module()
    print("module built OK")
